# revision 35
# baseline (speedup 1.0000x reference)
"""Block-sparse MoE (SwiGLU, top-k of 8 experts) on 8 Trainium2 NeuronCores.

Sharding: balanced expert-parallel (tensor-parallel over ffn_dim).
  - Routing (gate matmul + softmax + top-k, ~0.07% of total FLOPs) runs on
    the host; tokens are gathered per expert (full capacity, no drop).
  - EVERY core processes ALL 8 experts, but only a contiguous F/8 = 512
    slice of each expert's w1/w3/w2 rows.  Per-core work is therefore
    sum_e count_e * 3 * (F/8) * H MACs — identical on every core, i.e.
    perfectly load-balanced regardless of routing skew (the previous
    one-expert-per-core scheme paid max_e count_e ~ 8-9% padding).
  - Each core produces, per expert, a PARTIAL output y_c,e [H, C_e]
    (sum over its F-slice).  The host sums the 8 partials and
    scatter-adds into the full [T, H] output (partial sums commute with
    the routing-weight scale, which is applied on device).

Device kernel per core (per expert e, capacity C_e ~= count_e):
  phase 1: interT[fs, c] = silu(w1_s @ xT) * (w3_s @ xT)  fs in F-slice
  phase 2: yT[h, c]      = (w2_s.T @ interT) * wgt[c]     (partial over F/8)
Matmuls in fp16 (measured end-to-end rel err ~5e-4; fp32 weight loads
would make the PE weight-load-bound).  All DRAM->SBUF transfers are
host-pre-tiled so weight/x DMAs are contiguous.
"""

import math
import os

import numpy as np

H = 2048          # hidden dim
F = 4096          # ffn dim per expert
E = 8             # experts
NCORES = 8
P = 128           # partitions
NH = H // P       # 16 h-tiles
FS = F // NCORES  # 512: ffn slice per core
NFS = FS // P     # 4 f-tiles per (core, expert)

DT_MODE = os.environ.get("MOE_DT", "fp16")   # fp16 | bf16

# populated by kernel() for test harness introspection
LAST_STATS = {}

_BUILD_CACHE = {}


def _exp_chunks(count, force2=False):
    """Per-expert capacity C >= count split into even-width chunks <= 512
    (a matmul PSUM output tile must fit one 2 KB bank = 512 fp32)."""
    c = max(int(count), 2)
    c += c & 1
    n = max(1, math.ceil(c / 512))
    if force2:
        n = max(n, 2)
    w0 = 2 * math.ceil(c / (2 * n))
    return n * w0, [(i * w0, w0) for i in range(n)]


def _build(caps, dt_mode):
    """Build + compile the per-core Bass program.

    caps: tuple per expert of (C_e, w0_e, nch_e).  The same program runs
    SPMD on all 8 cores; the per-core difference is purely which F-slice
    of the weights the host placed in that core's input map.
    """
    import concourse.bacc as bacc
    import concourse.mybir as mybir
    from concourse import tile

    AF = mybir.ActivationFunctionType
    f32 = mybir.dt.float32
    dmm = mybir.dt.bfloat16 if dt_mode == "bf16" else mybir.dt.float16

    chunks_e = []
    for (C, w0, nch) in caps:
        chunks_e.append([(i * w0, w0) for i in range(nch)])
    CT = sum(c for (c, _, _) in caps)
    offs = np.cumsum([0] + [c for (c, _, _) in caps])[:-1]

    nc = bacc.Bacc("TRN2", target_bir_lowering=False, debug=False)

    # Host-pre-tiled DRAM layouts (per core):
    #   xt{e}  [nch, P, NH, w0]    xt[ci, p, n, c] = x_tok_e[ci*w0+c, n*P+p]
    #   w13t   [E*NFS, P, 2, NH, P] [e*NFS+fi, p, m, n, j]
    #                               = w{1,3}[e*F + core*FS + fi*P + j, n*P+p]
    #   w2t    [E*2, P, 8, NFS, P] [e*2+g, p, k, fi, j]
    #                       = w2[e*F + core*FS + fi*P + p, (g*8+k)*P + j]
    #   wgtb   [P, CT]             broadcast routing weights (all experts)
    #   y{e}   [P, NH, C_e]        partial out, y[p, n, c] = y_e[n*P+p, c]
    xt_d = [nc.dram_tensor(f"xt{e}", [caps[e][2], P, NH, caps[e][1]], dmm,
                           kind="ExternalInput").ap() for e in range(E)]
    w13_d = nc.dram_tensor("w13t", [E * NFS, P, 2, NH, P], dmm,
                           kind="ExternalInput").ap()
    w2_d = nc.dram_tensor("w2t", [E * 2, P, NH // 2, NFS, P], dmm,
                          kind="ExternalInput").ap()
    wg_d = nc.dram_tensor("wgtb", [P, CT], dmm, kind="ExternalInput").ap()
    y_d = [nc.dram_tensor(f"y{e}", [P, NH, caps[e][0]], dmm,
                          kind="ExternalOutput").ap() for e in range(E)]

    with tile.TileContext(nc) as tc:
        with (
            tc.tile_pool(name="inter", bufs=2) as inter_pool,
            tc.tile_pool(name="psum", bufs=2, space="PSUM") as psum_pool,
            tc.tile_pool(name="misc", bufs=1) as misc_pool,
            tc.tile_pool(name="xtp", bufs=4) as xt_pool,
            tc.tile_pool(name="wcol", bufs=5) as wcol_pool,
            tc.tile_pool(name="p1tmp", bufs=2) as p1tmp,
            tc.tile_pool(name="w2col", bufs=3) as w2_pool,
            tc.tile_pool(name="obp", bufs=4) as ob_pool,
        ):
            # PE warmup: zero-matmuls with no DMA dependencies run
            # immediately, lifting the HAM clock gate (1.2 -> 2.4 GHz)
            # while the first real loads are still in flight.
            wsrc = misc_pool.tile([P, P], dmm, tag="wsrc")
            nc.vector.memset(wsrc[:], 0.0)
            wps = psum_pool.tile([P, 64], f32, tag="ps3", bufs=4,
                                 name="warm_ps")
            for i in range(95):
                nc.tensor.matmul(wps[:], wsrc[:], wsrc[:, :64],
                                 start=(i == 0), stop=(i == 94))

            # Startup critical path: the SP queue issues its first DMA
            # earliest.  Interleave expert-0 chunk-a and the first weight
            # column in h-quarters so the first hi-tiles' matmuls start as
            # soon as the first quarters land.  Expert 0 is force-split
            # into 2 chunks so chunk-a is ~half the bytes.  All of w13(e0)
            # loads here; later experts' w13 is prefetched one expert ahead
            # (before the w2 loads of the current expert, which have more
            # slack).
            h4 = NH // 4
            xt_tiles = {}  # (e, ci) -> tile
            wc_tiles = {}  # (e, fi) -> tile
            for ci in range(caps[0][2]):
                xtc = xt_pool.tile([P, NH, caps[0][1]], dmm, tag="xt",
                                   name=f"xt0_{ci}")
                xt_tiles[(0, ci)] = xtc
            wc0 = wcol_pool.tile([P, 2, NH, P], dmm, tag="wc", name="wc0")
            wc_tiles[(0, 0)] = wc0
            for q in range(4):
                hs = slice(q * h4, (q + 1) * h4)
                nc.sync.dma_start(xt_tiles[(0, 0)][:, hs, :],
                                  xt_d[0][0][:, hs, :])
                nc.sync.dma_start(wc0[:, :, hs, :], w13_d[0][:, :, hs, :])
            wc_dmas = {}  # (e, fi) -> dma handle
            for fi in range(1, NFS):
                wc = wcol_pool.tile([P, 2, NH, P], dmm, tag="wc",
                                    name=f"wc0_{fi}")
                wc_tiles[(0, fi)] = wc
            # chunk-b and the f1 column interleave in h-halves: the f1
            # matmuls re-read both chunks, so neither transfer should
            # trail the other by a full megabyte
            h2 = NH // 2
            for hs in (slice(0, h2), slice(h2, NH)):
                for ci in range(1, caps[0][2]):
                    nc.sync.dma_start(xt_tiles[(0, ci)][:, hs, :],
                                      xt_d[0][ci][:, hs, :])
                wcd = nc.sync.dma_start(wc_tiles[(0, 1)][:, :, hs, :],
                                        w13_d[1][:, :, hs, :])
            wc_dmas[(0, 1)] = wcd
            # f2/f3 ride the otherwise-idle ACT HWDGE ring: splitting the
            # 6.3 MB startup weight stream across both rings lets chunk-b
            # and f1 land sooner on SP (no WAR waits here, so the ACT
            # queue never blocks the sigmoids behind it)
            for fi in range(2, NFS):
                wc_dmas[(0, fi)] = nc.scalar.dma_start(wc_tiles[(0, fi)][:],
                                                       w13_d[fi])

            # routing weights ride the otherwise-idle ACT HWDGE ring, gated
            # past the startup-critical first w13 columns (needed at the
            # phase-2(e0) ob-muls ~44us in; arriving late stalls the po
            # PSUM ring and with it the whole PE).
            wgtb = misc_pool.tile([P, CT], dmm, tag="wgtb")
            wgtb_dma = nc.scalar.dma_start(wgtb[:], wg_d[:])
            tile.add_dep_helper(wgtb_dma.ins, wc_dmas[(0, 2)].ins,
                                reason="delay wgtb load past startup")

            for e in range(E):
                C, w0, nch = caps[e]
                chunks = chunks_e[e]
                off = int(offs[e])

                # This expert's w2 halves first on the SP ring (they are
                # WAR-free at block start and needed at phase-2 start; two
                # batched 1 MB DMAs amortize the per-DMA HWDGE latency that
                # starved phase 2 when loaded per h-tile).  Then the next
                # expert's w13 columns, whose pool WAR gates trickle them
                # out as phase 1 progresses.  Token tiles ride the SWDGE
                # (gpsimd) ring — a pool-slot wait there only delays output
                # stores, never the sigmoids (ACT) or weight loads (SP)
                # the PE depends on.
                w2g_tiles = []
                for g in range(2):
                    w2g = w2_pool.tile([P, NH // 2, NFS, P], dmm, tag="w2c",
                                       name=f"w2g{e}_{g}")
                    w2g_tiles.append(w2g)
                    nc.sync.dma_start(w2g[:], w2_d[e * 2 + g])
                if e + 1 < E:
                    for fi in range(NFS):
                        wc = wcol_pool.tile([P, 2, NH, P], dmm, tag="wc",
                                            name=f"wc{e + 1}_{fi}")
                        wc_tiles[(e + 1, fi)] = wc
                        wc_dmas[(e + 1, fi)] = nc.sync.dma_start(
                            wc[:], w13_d[(e + 1) * NFS + fi])
                    for ci in range(caps[e + 1][2]):
                        xtc = xt_pool.tile([P, NH, caps[e + 1][1]], dmm,
                                           tag="xt", name=f"xt{e + 1}_{ci}")
                        xt_tiles[(e + 1, ci)] = xtc
                        xtd = nc.gpsimd.dma_start(xtc[:], xt_d[e + 1][ci])
                        # keep SWDGE token prefetch off the startup window
                        # (the xt pool ring throttles the lookahead; the
                        # scheduler hoists phase-1(e+1) matmuls ahead of
                        # the phase-2(e) tail, so this must land early)
                        tile.add_dep_helper(
                            xtd.ins, wc_dmas[(e, 1)].ins,
                            reason="stagger xt prefetch behind w13")


                # ---- phase 1: interT = silu(w1s @ xT) * (w3s @ xT) ----
                inter_tiles = []
                for fi in range(NFS):
                    wc = wc_tiles[(e, fi)]
                    it = inter_pool.tile([P, C], dmm, tag=f"inter{fi}",
                                         name=f"inter{e}_{fi}")
                    inter_tiles.append(it)
                    ps1 = [psum_pool.tile([P, cw], f32, tag="ps1", bufs=4,
                                          name=f"ps1_{e}_{fi}_{ci}")
                           for ci, (c0, cw) in enumerate(chunks)]
                    ps3 = [psum_pool.tile([P, cw], f32, tag="ps3", bufs=4,
                                          name=f"ps3_{e}_{fi}_{ci}")
                           for ci, (c0, cw) in enumerate(chunks)]
                    if e == 0 and fi == 0:
                        # chunk-sequential: chunk-a's matmuls depend only
                        # on the first (half) xt DMA
                        for ci in range(nch):
                            for hi in range(NH):
                                nc.tensor.matmul(
                                    ps1[ci][:], wc[:, 0, hi, :],
                                    xt_tiles[(e, ci)][:, hi, :],
                                    start=(hi == 0), stop=(hi == NH - 1))
                            for hi in range(NH):
                                nc.tensor.matmul(
                                    ps3[ci][:], wc[:, 1, hi, :],
                                    xt_tiles[(e, ci)][:, hi, :],
                                    start=(hi == 0), stop=(hi == NH - 1))
                    else:
                        # interleaved: consecutive matmuls share the
                        # stationary operand across chunks
                        for hi in range(NH):
                            for ci in range(nch):
                                nc.tensor.matmul(
                                    ps1[ci][:], wc[:, 0, hi, :],
                                    xt_tiles[(e, ci)][:, hi, :],
                                    start=(hi == 0), stop=(hi == NH - 1))
                            for ci in range(nch):
                                nc.tensor.matmul(
                                    ps3[ci][:], wc[:, 1, hi, :],
                                    xt_tiles[(e, ci)][:, hi, :],
                                    start=(hi == 0), stop=(hi == NH - 1))
                    for ci, (c0, cw) in enumerate(chunks):
                        # silu(a) = a * sigmoid(a)
                        sig = p1tmp.tile([P, cw], f32, tag="sig")
                        nc.scalar.activation(sig[:], ps1[ci][:], AF.Sigmoid)
                        sil = p1tmp.tile([P, cw], f32, tag="sil")
                        nc.vector.tensor_mul(sil[:], ps1[ci][:], sig[:])
                        nc.vector.tensor_mul(it[:, c0:c0 + cw], sil[:],
                                             ps3[ci][:])

                # ---- phase 2: yT[ht, :] = (w2s.T @ interT) * wgt ----
                # w2 streams on the SP ring behind the next expert's w13
                # prefetch; output stores go out on the otherwise-idle SWDGE
                # (gpsimd) path in quarter (4 h-tile) granularity.
                for ht in range(NH):
                    if ht % 4 == 0:
                        ob = ob_pool.tile([P, 4, C], dmm, tag="ob",
                                          name=f"ob{e}_{ht // 4}")
                    w2c = w2g_tiles[ht // 8]
                    po = [psum_pool.tile([P, cw], f32, tag="ps1", bufs=4,
                                         name=f"po_{e}_{ht}_{ci}")
                          for ci, (c0, cw) in enumerate(chunks)]
                    for fi in range(NFS):
                        for ci, (c0, cw) in enumerate(chunks):
                            nc.tensor.matmul(
                                po[ci][:], w2c[:, ht % 8, fi, :],
                                inter_tiles[fi][:, c0:c0 + cw],
                                start=(fi == 0), stop=(fi == NFS - 1))
                    for ci, (c0, cw) in enumerate(chunks):
                        nc.vector.tensor_mul(
                            ob[:, ht % 4, c0:c0 + cw], po[ci][:],
                            wgtb[:, off + c0:off + c0 + cw])
                    if e == E - 1 and ht >= NH - 4:
                        # final expert: per-h-tile stores on the (now idle)
                        # SP HWDGE ring, so the last store is small and the
                        # SWDGE drain at kernel exit has nothing pending
                        nc.sync.dma_start(
                            y_d[e][:, ht:ht + 1, :], ob[:, ht % 4:ht % 4 + 1])
                    elif ht % 4 == 3:
                        ring = nc.sync if e == E - 1 else nc.gpsimd
                        ring.dma_start(
                            y_d[e][:, ht - 3:ht + 1, :], ob[:])

    nc.compile()
    return nc


def _get_nc(caps, dt_mode):
    key = (caps, dt_mode)
    if key not in _BUILD_CACHE:
        _BUILD_CACHE[key] = _build(caps, dt_mode)
    return _BUILD_CACHE[key]


def _route(x, gate_w, top_k):
    """Host routing, matching the reference exactly:
    softmax(x @ gate_w.T) -> top-k (ties -> lower index) -> renormalize."""
    logits = x.astype(np.float64) @ gate_w.astype(np.float64).T
    m = logits.max(axis=-1, keepdims=True)
    p = np.exp(logits - m)
    p /= p.sum(axis=-1, keepdims=True)
    idx = np.argsort(-p, axis=-1, kind="stable")[:, :top_k]          # [T, k]
    vals = np.take_along_axis(p, idx, axis=-1)
    vals = vals / vals.sum(axis=-1, keepdims=True)
    return idx, vals.astype(np.float32)


def _fake_device(in_maps, caps):
    """Numpy stand-in for the device: consumes the exact tiled in_maps
    (validates host-side layouts end-to-end). Dev aid, off by default."""
    class R:
        exec_time_ns = None
        mean_exec_time_ns = None
        instructions_and_trace = None
        profile_json = None
        results = []
    res = R()
    for m in in_maps:
        out = {}
        for e in range(E):
            C, w0, nch = caps[e]
            xs = m[f"xt{e}"].transpose(0, 3, 2, 1).reshape(C, H).astype(
                np.float32)
            w13 = m["w13t"][e * NFS:(e + 1) * NFS]        # [4, P, 2, NH, P]
            w1s = w13[:, :, 0].transpose(0, 3, 2, 1).reshape(FS, H).astype(
                np.float32)
            w3s = w13[:, :, 1].transpose(0, 3, 2, 1).reshape(FS, H).astype(
                np.float32)
            w2a = m["w2t"][e * 2:(e + 1) * 2]         # [2, P, 8, NFS, P]
            w2s = w2a.transpose(3, 1, 0, 2, 4).reshape(FS, H).astype(
                np.float32)
            off = sum(c for (c, _, _) in caps[:e])
            wgt = m["wgtb"][0, off:off + C]
            h1 = xs @ w1s.T
            h3 = xs @ w3s.T
            inter = (h1 / (1 + np.exp(-h1))) * h3
            ye = (inter @ w2s) * wgt[:, None]             # [C, H]
            out[f"y{e}"] = np.ascontiguousarray(
                ye.T.reshape(NH, P, C).transpose(1, 0, 2))
        res.results.append(out)
    return res


def kernel(x, gate_w, w1, w2, w3, top_k):
    x = np.ascontiguousarray(np.asarray(x, dtype=np.float32))
    gate_w = np.asarray(gate_w, dtype=np.float32)
    w1 = np.asarray(w1, dtype=np.float32)
    w2 = np.asarray(w2, dtype=np.float32)
    w3 = np.asarray(w3, dtype=np.float32)
    k = int(np.asarray(top_k))
    t, h = x.shape
    e = gate_w.shape[0]
    f = w1.shape[0] // e
    assert (h, f, e) == (H, F, E), (h, f, e)

    dt_mode = DT_MODE
    import ml_dtypes
    np_mm = {"bf16": ml_dtypes.bfloat16}.get(dt_mode, np.float16)

    idx, vals = _route(x, gate_w, k)                                  # [T, k]

    # token lists per expert
    tok_lists = []
    wgt_lists = []
    for ei in range(E):
        tok_i, slot_i = np.nonzero(idx == ei)
        tok_lists.append(tok_i.astype(np.int64))
        wgt_lists.append(vals[tok_i, slot_i].astype(np.float32))
    caps = []
    for ei in range(E):
        C, chunks = _exp_chunks(len(tok_lists[ei]), force2=(ei == 0))
        caps.append((C, chunks[0][1], len(chunks)))
    caps = tuple(caps)
    CT = sum(c for (c, _, _) in caps)

    xmm = x.astype(np_mm)

    # xt / wgtb are identical for all cores (weights differ per core)
    xts = []
    wgt_full = np.zeros(CT, dtype=np.float32)
    off = 0
    for ei in range(E):
        C, w0, nch = caps[ei]
        tok = tok_lists[ei]
        n = len(tok)
        xs = np.zeros((C, H), dtype=np_mm)
        xs[:n] = xmm[tok]
        xts.append(np.ascontiguousarray(
            xs.reshape(nch, w0, NH, P).transpose(0, 3, 2, 1)))
        wgt_full[off:off + n] = wgt_lists[ei]
        off += C
    wgtb = np.ascontiguousarray(
        np.broadcast_to(wgt_full, (P, CT)).astype(np_mm))

    # per-core weight slices:
    #   w13_all [cs, e, fi, p, m, n, j];  w2_all [cs, e, ht, p, fi, j]
    w1r = w1.reshape(E, NCORES, NFS, P, NH, P)      # [e, cs, fi, j, n, p]
    w3r = w3.reshape(E, NCORES, NFS, P, NH, P)
    w2r = w2.reshape(E, NCORES, NFS, P, NH, P)      # [e, cs, fi, p, ht, j]
    w13_all = np.empty((NCORES, E, NFS, P, 2, NH, P), dtype=np_mm)
    w13_all[:, :, :, :, 0] = w1r.transpose(1, 0, 2, 5, 4, 3).astype(np_mm)
    w13_all[:, :, :, :, 1] = w3r.transpose(1, 0, 2, 5, 4, 3).astype(np_mm)
    w2g = w2r.reshape(E, NCORES, NFS, P, 2, NH // 2, P)
    w2_all = np.ascontiguousarray(
        w2g.transpose(1, 0, 4, 3, 5, 2, 6).astype(np_mm))

    in_maps = []
    for c in range(NCORES):
        m = {"w13t": w13_all[c].reshape(E * NFS, P, 2, NH, P),
             "w2t": w2_all[c].reshape(E * 2, P, NH // 2, NFS, P),
             "wgtb": wgtb}
        for ei in range(E):
            m[f"xt{ei}"] = xts[ei]
        in_maps.append(m)

    if os.environ.get("MOE_FAKE"):
        res = _fake_device(in_maps, caps)
    else:
        from concourse.bass_utils import run_bass_kernel_spmd
        nc = _get_nc(caps, dt_mode)
        trace = bool(int(os.environ.get("MOE_TRACE", "0")))
        res = run_bass_kernel_spmd(nc, in_maps, core_ids=list(range(NCORES)),
                                   trace=trace)
    LAST_STATS.clear()
    LAST_STATS.update({
        "caps": caps,
        "dt_mode": dt_mode,
        "exec_time_ns": res.exec_time_ns,
        "mean_exec_time_ns": res.mean_exec_time_ns,
        "counts": [len(ti) for ti in tok_lists],
        "trace": getattr(res, "instructions_and_trace", None) and
                 res.instructions_and_trace[1],
        "profile_json": getattr(res, "profile_json", None),
    })

    out = np.zeros((t, h), dtype=np.float32)
    for ei in range(E):
        n = len(tok_lists[ei])
        C = caps[ei][0]
        acc = np.zeros((P, NH, C), dtype=np.float32)
        for c in range(NCORES):
            acc += res.results[c][f"y{ei}"].astype(np.float32)
        # y[p, nh, c] -> [c, h] with h = nh*P + p
        ye = acc.transpose(2, 1, 0).reshape(C, H)
        out[tok_lists[ei]] += ye[:n]
    return out
